# revision 1
# baseline (speedup 1.0000x reference)
"""Trainium2 Bass kernel for nn_MultiLIF_17059610100026.

Adaptive LIF neuron layer: for input I[B=32, L=1024, K=512], runs the
per-(b,k) time recurrence

    th     = 1.5 + 1.5*a
    v_pre  = 0.95*v + I_t
    s      = (v_pre >= th)
    sn    += s
    v      = s ? -0.5 : v_pre
    a      = 0.99*a + s

and returns (spikes, series, v_seq), each [B, L, K] f32.

Sharding: fully data-parallel over B — core c gets b in [4c, 4c+4).

Per-core design (time blocks of T=128):
 - DMA uses the partition-major pattern (partition = time row, 2KB
   contiguous per partition) which is the only fast DGE pattern here.
 - Layout conversion staging [tau, k] <-> compute [k % 128, (b, kh), tau]
   is done with PE transposes (via an identity matrix) into PSUM and
   ACT copies back to SBUF — both off the critical DVE path.
 - The serial recurrence runs on DVE as 6 fused ops per step on the
   whole per-core state ([128, 16] = 2048 neurons).
 - `series` comes from one segmented tensor_tensor_scan per block;
   the u8 spike mask is cast to f32 on ACT.
"""
import numpy as np

B, L, K = 32, 1024, 512
NCORES = 8
B_LOC = B // NCORES          # 4
P = 128                      # partitions
KH = K // P                  # 4 k-groups
NN = B_LOC * KH              # 16 neurons per partition
T = 128                      # time block
NBLK = L // T

_cache = {}


def _legalize_waits(nc, max_waits=1):
    """Split multi-wait instructions into chains of single-wait NoOps.

    The walrus build here rejects instructions carrying more than one
    sync-wait. Hoist extra waits onto NoOps on the same engine right
    before the instruction (engines execute in order, so this is
    semantically identical).
    """
    import concourse.mybir as mybir

    n = 0
    ctr = [0]
    for fn in nc.m.functions:
        for blk in fn.blocks:
            insts = list(blk.instructions)
            out = []
            changed = False
            for ins in insts:
                si = ins.sync_info
                waits = list(si.on_wait) if (si is not None and si.on_wait) else []
                if len(waits) > max_waits:
                    for w in waits[max_waits:]:
                        ctr[0] += 1
                        nop = mybir.InstNoOp(name=f"legal-wait-nop-{ctr[0]}")
                        nop.engine = ins.engine
                        nop.sync_info = mybir.SyncInfo(on_wait=[w], on_update=[])
                        out.append(nop)
                    ins.sync_info = mybir.SyncInfo(
                        on_wait=waits[:max_waits],
                        on_update=list(si.on_update or []),
                    )
                    changed = True
                    n += 1
                out.append(ins)
            if changed:
                blk.instructions = out
    return n


def _build(nblk=NBLK, reps=1):
    import concourse.bass as bass
    import concourse.mybir as mybir
    from concourse.tile import TileContext

    f32 = mybir.dt.float32
    u8 = mybir.dt.uint8
    A = mybir.AluOpType

    nc = bass.Bass()
    I_d = nc.declare_dram_parameter("I", [B_LOC, L, K], f32, isOutput=False)
    spk_d = nc.declare_dram_parameter("spikes", [B_LOC, L, K], f32, isOutput=True)
    ser_d = nc.declare_dram_parameter("series", [B_LOC, L, K], f32, isOutput=True)
    vsq_d = nc.declare_dram_parameter("v_seq", [B_LOC, L, K], f32, isOutput=True)

    with TileContext(nc) as tc:
        with (
            tc.tile_pool(name="state", bufs=1) as stp,
            tc.tile_pool(name="io", bufs=2) as iop,
            tc.tile_pool(name="ps", bufs=2, space="PSUM") as psp,
        ):
            v_post = stp.tile([P, NN], f32, name="v_post", tag="v_post")
            a = stp.tile([P, NN], f32, name="a", tag="a")
            sn_carry = stp.tile([P, NN], f32, name="sn_carry", tag="sn_carry")
            neghalf = stp.tile([P, NN], f32, name="neghalf", tag="neghalf")
            th = stp.tile([P, NN], f32, name="th", tag="th")
            d0sn = stp.tile([P, NN * T], f32, name="d0sn", tag="d0sn")
            ident = stp.tile([P, P], f32, name="ident", tag="ident")
            ones = stp.tile([P, P], f32, name="ones", tag="ones")

            nc.vector.memset(v_post[:], 0.0)
            nc.vector.memset(a[:], 0.0)
            nc.vector.memset(sn_carry[:], 0.0)
            nc.vector.memset(neghalf[:], -0.5)
            nc.vector.memset(d0sn[:], 1.0)
            d0v = d0sn[:].rearrange("p (n t) -> p n t", t=T)
            nc.vector.memset(d0v[:, :, 0:1], 0.0)
            nc.vector.memset(ones[:], 1.0)
            nc.gpsimd.affine_select(
                out=ident[:], in_=ones[:], pattern=[[-1, P]], base=0,
                channel_multiplier=1, compare_op=A.is_equal, fill=0.0)

            for _ri in range(reps * nblk):
                blk = _ri % nblk
                if blk == 0 and _ri > 0:
                    # benchmarking only (reps > 1): reset state per repetition
                    nc.vector.memset(v_post[:], 0.0)
                    nc.vector.memset(a[:], 0.0)
                    nc.vector.memset(sn_carry[:], 0.0)
                Xg = iop.tile([P, B_LOC * K], f32, name="Xg", tag="Xg")
                Xi = iop.tile([P, NN * T], f32, name="Xi", tag="Xi")
                Vst = iop.tile([P, NN * T], f32, name="Vst", tag="Vst")
                Vg = iop.tile([P, B_LOC * K], f32, name="Vg", tag="Vg")
                S8 = iop.tile([P, NN * T], u8, name="S8", tag="S8")
                Sf = iop.tile([P, NN * T], f32, name="Sf", tag="Sf")
                Sg = iop.tile([P, B_LOC * K], f32, name="Sg", tag="Sg")
                SN = iop.tile([P, NN * T], f32, name="SN", tag="SN")
                SNg = iop.tile([P, B_LOC * K], f32, name="SNg", tag="SNg")

                Xgv = Xg[:].rearrange("p (b k) -> p b k", b=B_LOC)
                Xiv = Xi[:].rearrange("p (n t) -> p n t", t=T)
                for b in range(B_LOC):
                    nc.sync.dma_start(out=Xgv[:, b],
                                      in_=I_d[b, blk * T:(blk + 1) * T, :])
                # staging [tau, k] -> compute [k%P, n=(b,kh), tau]
                for b in range(B_LOC):
                    for kh in range(KH):
                        pin = psp.tile([P, P], f32, name="pin", tag="pin")
                        nc.tensor.transpose(
                            pin[:], Xgv[:, b, kh * P:(kh + 1) * P], ident[:])
                        nc.scalar.copy(out=Xiv[:, b * KH + kh], in_=pin[:])

                Vv = Vst[:].rearrange("p (n t) -> p n t", t=T)
                S8v = S8[:].rearrange("p (n t) -> p n t", t=T)
                for tau in range(T):
                    nc.vector.scalar_tensor_tensor(
                        out=Vv[:, :, tau], in0=v_post[:], scalar=0.95,
                        in1=Xiv[:, :, tau], op0=A.mult, op1=A.add)
                    nc.vector.tensor_scalar(
                        out=th[:], in0=a[:], scalar1=1.5, scalar2=1.5,
                        op0=A.mult, op1=A.add)
                    nc.vector.tensor_tensor(
                        out=S8v[:, :, tau], in0=Vv[:, :, tau], in1=th[:],
                        op=A.is_ge)
                    nc.vector.tensor_copy(out=v_post[:], in_=Vv[:, :, tau])
                    nc.vector.copy_predicated(
                        out=v_post[:], mask=S8v[:, :, tau], data=neghalf[:])
                    nc.vector.scalar_tensor_tensor(
                        out=a[:], in0=a[:], scalar=0.99,
                        in1=S8v[:, :, tau], op0=A.mult, op1=A.add)

                # spikes: cast u8 -> f32 (ACT)
                nc.scalar.copy(out=Sf[:], in_=S8[:])
                Sfv = Sf[:].rearrange("p (n t) -> p n t", t=T)

                # v_seq + spikes out: compute -> PSUM transpose -> staging -> DRAM
                Vgv = Vg[:].rearrange("p (b k) -> p b k", b=B_LOC)
                Sgv = Sg[:].rearrange("p (b k) -> p b k", b=B_LOC)
                for b in range(B_LOC):
                    for kh in range(KH):
                        n = b * KH + kh
                        pv = psp.tile([P, P], f32, name="pv", tag="pv")
                        nc.tensor.transpose(pv[:], Vv[:, n], ident[:])
                        nc.scalar.copy(out=Vgv[:, b, kh * P:(kh + 1) * P], in_=pv[:])
                        pso = psp.tile([P, P], f32, name="pso", tag="pso")
                        nc.tensor.transpose(pso[:], Sfv[:, n], ident[:])
                        nc.scalar.copy(out=Sgv[:, b, kh * P:(kh + 1) * P], in_=pso[:])
                for b in range(B_LOC):
                    nc.sync.dma_start(out=vsq_d[b, blk * T:(blk + 1) * T, :],
                                      in_=Vgv[:, b])
                    nc.sync.dma_start(out=spk_d[b, blk * T:(blk + 1) * T, :],
                                      in_=Sgv[:, b])

                # series: carry into col 0 (after spike transposes read Sf),
                # segmented prefix-sum scan, then transpose out
                nc.vector.tensor_tensor(
                    out=Sfv[:, :, 0], in0=Sfv[:, :, 0], in1=sn_carry[:],
                    op=A.add)
                nc.vector.tensor_tensor_scan(
                    out=SN[:], data0=d0sn[:], data1=Sf[:], initial=0.0,
                    op0=A.mult, op1=A.add)
                SNv = SN[:].rearrange("p (n t) -> p n t", t=T)
                nc.scalar.copy(out=sn_carry[:], in_=SNv[:, :, T - 1])
                SNgv = SNg[:].rearrange("p (b k) -> p b k", b=B_LOC)
                for b in range(B_LOC):
                    for kh in range(KH):
                        n = b * KH + kh
                        psn = psp.tile([P, P], f32, name="psn", tag="psn")
                        nc.tensor.transpose(psn[:], SNv[:, n], ident[:])
                        nc.scalar.copy(out=SNgv[:, b, kh * P:(kh + 1) * P], in_=psn[:])
                for b in range(B_LOC):
                    nc.sync.dma_start(out=ser_d[b, blk * T:(blk + 1) * T, :],
                                      in_=SNgv[:, b])

    _legalize_waits(nc)
    return nc


def kernel(I, _nblk=NBLK):
    from concourse.bass_utils import run_bass_kernel_spmd

    I = np.ascontiguousarray(np.asarray(I, dtype=np.float32))
    if _nblk not in _cache:
        _cache[_nblk] = _build(_nblk)
    nc = _cache[_nblk]

    in_maps = [{"I": I[c * B_LOC:(c + 1) * B_LOC]} for c in range(NCORES)]
    out = run_bass_kernel_spmd(nc, in_maps, list(range(NCORES)))
    res = out.results
    spikes = np.concatenate([res[c]["spikes"] for c in range(NCORES)], axis=0)
    series = np.concatenate([res[c]["series"] for c in range(NCORES)], axis=0)
    v_seq = np.concatenate([res[c]["v_seq"] for c in range(NCORES)], axis=0)
    return spikes, series, v_seq

